# revision 22
# baseline (speedup 1.0000x reference)
"""Trainium2 Bass kernel: varlen batched cross-attention (sparse_attention).

Math (per reference):
  qh = q @ Wq.T           [Tq, H, D]
  k,v = split(x @ Wkv.T)  [B, N, H, D]
  per batch b: queries of segment b attend over batch b's N keys
  out = softmax(qh k^T / sqrt(D)) v  -> [Tq, C] @ Wproj.T + bproj

Design:
  Load-balanced 3-slot sharding: every core processes 1536 query slots
  (12 tiles of 128): slot A = first 1024 queries of its home batch,
  slots B, C = 256-query chunks of longer batches' remainders.  Every
  core computes K,V for its 3 slot batches locally; zero collectives;
  one uniform NEFF; host scatters inputs / gathers outputs.

  Scores run on the PE in plain fp8e4 (64-dim contraction, heads packed
  two-per-128-partition tile, tile_position row offset 0/64).  Wq/Wk
  are host-scaled by 8 for fp8 range (exp scale folds 1/64 back); K^T
  and qh^T are cast f32->fp8 from the projection PSUM on the DVE.
  (fp8 DoubleRow was tried and measured ~2x SLOWER per column than
  plain fp8/bf16 on real hardware, despite the cost model's 0.5
  cycles/row - do not bring it back without re-measuring.)

  exp() on ACT is the stream bottleneck (~1.53us per [128,1536] chunk,
  128 chunks): single-pass over (head, keytile), PSUM chunk ring
  2x[128,1536] (6 banks) + two po accumulators (2 banks) = 8 banks.
  Score matmuls must not cross 512-col PSUM bank boundaries.

  AV uses the P-stationary form: po[qtile, 65] += P~[kt, qtile]^T @
  V_aug[kt]; the V ones-column yields Z for free; 16-keytile PSUM
  accumulation uses memset + start=False multi-group banks.  Normalize
  = DVE recip + broadcast-mult into O [q, feat] bf16; O -> O^T via
  DMA-engine transpose (sync queue ONLY - issuing dma_start_transpose
  from the ACT queue races and corrupts results); y^T = Wp^T O^T +
  bias in 512-col chunks, DMA out [C, 1536] f32 per core.

  Prologue (~110us, ACT idle): all K projections + q8 + V leads; the
  remaining V_b/V_c units drain one-per-window into the first stream
  windows.  Moving MORE projection work into stream windows (v6
  experiment) serialized the stream through the PSUM ring and was
  ~140us slower - the ring FIFO couples background tiles to stream
  chunks, so background work per window must stay under the ~500ns
  ACT slack.
"""

import os
import numpy as np

B, NKEY, C, H, D = 8, 2048, 512, 8, 64
NCORES = 8
CT = C // 128           # 4 c-tiles
NT = NKEY // 128        # 16 key tiles
L = 1536                # uniform padded queries per core
NQT = L // 128          # 12 query tiles
ALEN = 1024             # slot A capacity (tiles 0-7)
CHUNK = 256             # slot B/C capacity (2 tiles each)
SCALE = float(D) ** -0.5
W8 = 8.0                # host pre-scale on Wq/Wk for fp8 range
EXPSCALE = SCALE / (W8 * W8)
P1W = 896               # pass1 width: tiles 0-6
P2W = 640               # pass2 width: tile 7 (128) + B (256) + C (256)

_BUILD_CACHE = {}


def _build():
    if "nc" in _BUILD_CACHE:
        return _BUILD_CACHE["nc"]
    from contextlib import ExitStack
    import concourse.bass as bass
    import concourse.tile as tile
    import concourse.mybir as mybir
    from concourse import bacc

    f32 = mybir.dt.float32
    bf16 = mybir.dt.bfloat16
    fp8 = mybir.dt.float8e4
    AF = mybir.ActivationFunctionType
    ALU = mybir.AluOpType
    DR = mybir.MatmulPerfMode.DoubleRow

    nc = bacc.Bacc("TRN2", target_bir_lowering=False, debug=False)
    xT = {s: nc.declare_dram_parameter(f"xT{s}", [C, NKEY], bf16, isOutput=False)
          for s in "abc"}
    qT = nc.declare_dram_parameter("qT", [C, L], bf16, isOutput=False)
    wqT = nc.declare_dram_parameter("wqT", [C, C], bf16, isOutput=False)   # permuted, x8
    wkT = nc.declare_dram_parameter("wkT", [C, C], bf16, isOutput=False)   # permuted, x8
    wvT = nc.declare_dram_parameter("wvT", [C, C], bf16, isOutput=False)
    wpT = nc.declare_dram_parameter("wpT", [C, C], bf16, isOutput=False)
    biasP = nc.declare_dram_parameter("biasP", [128, CT], f32, isOutput=False)
    outT = nc.declare_dram_parameter("out", [C, L], f32, isOutput=True)

    with ExitStack() as ctx:
        tc = ctx.enter_context(tile.TileContext(nc))
        pers = ctx.enter_context(tc.tile_pool(name="pers", bufs=1))
        psS = ctx.enter_context(tc.tile_pool(name="psS", bufs=2, space="PSUM"))
        psO = ctx.enter_context(tc.tile_pool(name="psO", bufs=1, space="PSUM"))
        pts = ctx.enter_context(tc.tile_pool(name="pts", bufs=3))
        work = ctx.enter_context(tc.tile_pool(name="work", bufs=2))
        qot = ctx.enter_context(tc.tile_pool(name="qot", bufs=4))

        # ---- persistent inputs -------------------------------------------
        wq_sb = [pers.tile([128, C], bf16, tag=f"wq{i}", name=f"wq{i}") for i in range(CT)]
        wk_sb = [pers.tile([128, C], bf16, tag=f"wk{i}", name=f"wk{i}") for i in range(CT)]
        wv_sb = [pers.tile([128, C], bf16, tag=f"wv{i}", name=f"wv{i}") for i in range(CT)]
        wp_sb = [pers.tile([128, C], bf16, tag=f"wp{i}", name=f"wp{i}") for i in range(CT)]
        xs_sb = {s: [pers.tile([128, NKEY], bf16, tag=f"x{s}{i}", name=f"x{s}{i}")
                     for i in range(CT)] for s in "abc"}
        # qt tiles share a rotating pool with OT (disjoint lifetimes)
        qt_sb = [qot.tile([128, L], bf16, tag="qot", name=f"qt{i}") for i in range(CT)]
        bias_sb = pers.tile([128, CT], f32, tag="bias")

        for i in range(CT):
            sl = slice(128 * i, 128 * (i + 1))
            nc.sync.dma_start(wq_sb[i][:], wqT[sl, :])
            nc.sync.dma_start(qt_sb[i][:], qT[sl, :])
            nc.sync.dma_start(wk_sb[i][:], wkT[sl, :])
            nc.sync.dma_start(xs_sb["a"][i][:], xT["a"][sl, :])
            nc.sync.dma_start(wv_sb[i][:], wvT[sl, :])
        for i in range(CT):
            sl = slice(128 * i, 128 * (i + 1))
            nc.sync.dma_start(xs_sb["b"][i][:], xT["b"][sl, :])
            nc.sync.dma_start(xs_sb["c"][i][:], xT["c"][sl, :])
            nc.sync.dma_start(wp_sb[i][:], wpT[sl, :])
        nc.sync.dma_start(bias_sb[:], biasP[:])

        # fp8 operands, natural layout: tile jt = heads (2jt, 2jt+1) x d(64)
        q8 = [pers.tile([128, L], fp8, tag=f"q8{t}", name=f"q8{t}") for t in range(CT)]
        k8 = {s: [pers.tile([128, NKEY], fp8, tag=f"k8{s}{t}", name=f"k8{s}{t}")
                  for t in range(CT)] for s in "abc"}
        va = {s: [pers.tile([128, H * (D + 1)], bf16, tag=f"va{s}{n}", name=f"va{s}{n}")
                  for n in range(NT)] for s in "abc"}

        # ---- projections (psum staged in the shared psS ring) -----------
        def proj_q8(jt):
            ps = psS.tile([128, L], f32, tag="psS")
            for lcs in range(0, L, 512):
                for ct in range(CT):
                    nc.tensor.matmul(
                        ps[:, lcs:lcs + 512],
                        lhsT=wq_sb[ct][:, 128 * jt:128 * (jt + 1)],
                        rhs=qt_sb[ct][:, lcs:lcs + 512],
                        start=(ct == 0), stop=(ct == CT - 1),
                        skip_group_check=True)
            nc.vector.tensor_copy(q8[jt][:, 0:L], ps[:, 0:L])

        def proj_k8(s, jt, part):
            # part 0: key cols 0:1536 (3 groups); part 1: cols 1536:2048
            if part == 0:
                ps = psS.tile([128, L], f32, tag="psS")
                for g in range(3):
                    for ct in range(CT):
                        nc.tensor.matmul(
                            ps[:, 512 * g:512 * (g + 1)],
                            lhsT=wk_sb[ct][:, 128 * jt:128 * (jt + 1)],
                            rhs=xs_sb[s][ct][:, 512 * g:512 * (g + 1)],
                            start=(ct == 0), stop=(ct == CT - 1),
                            skip_group_check=True)
                nc.vector.tensor_copy(k8[s][jt][:, 0:1536], ps[:, 0:1536])
            else:
                ps = psS.tile([128, L], f32, tag="psS")
                for ct in range(CT):
                    nc.tensor.matmul(
                        ps[:, 0:512],
                        lhsT=wk_sb[ct][:, 128 * jt:128 * (jt + 1)],
                        rhs=xs_sb[s][ct][:, 1536:2048],
                        start=(ct == 0), stop=(ct == CT - 1),
                        skip_group_check=True)
                nc.vector.tensor_copy(k8[s][jt][:, 1536:2048], ps[:, 0:512])

        def proj_v(s, nt, on_act):
            ps = psS.tile([128, L], f32, tag="psS")
            for ct in range(CT):
                nc.tensor.matmul(
                    ps[:, 0:512],
                    lhsT=xs_sb[s][ct][:, 128 * nt:128 * (nt + 1)],
                    rhs=wv_sb[ct][:, 0:C],
                    start=(ct == 0), stop=(ct == CT - 1),
                    skip_group_check=True)
            va3 = va[s][nt][:].rearrange("p (h e) -> p h e", h=H)
            ps3 = ps[:, 0:512].rearrange("p (h d) -> p h d", h=H)
            if on_act:
                nc.scalar.copy(va3[:, :, 0:D], ps3[:, :, :])
            else:
                nc.vector.tensor_copy(va3[:, :, 0:D], ps3[:, :, :])
            nc.vector.memset(va3[:, :, D:D + 1], 1.0)

        # ---- attention ----------------------------------------------------
        o_big = pers.tile([128, NQT, C], bf16, tag="obig")
        o3 = o_big[:]

        # single-pass chunk layout (none may cross a 512-col psum bank):
        SC_PARTS = [(0, 512, "a", 0), (512, 512, "a", 512),
                    (1024, 256, "b", 1024), (1280, 256, "c", 1280)]
        AV_G1 = [(g, 128 * g, "a") for g in range(7)]               # tiles 0-6
        AV_G2 = [(0, 896, "a"), (1, 1024, "b"), (2, 1152, "b"),
                 (3, 1280, "c"), (4, 1408, "c")]                    # tiles 7-11

        def scores(ps, hh, kt):
            jt, off = hh // 2, 64 * (hh % 2)
            for (cs, cn, s, qs) in SC_PARTS:
                nc.tensor.matmul(
                    ps[:, cs:cs + cn],
                    lhsT=k8[s][jt][off:off + 64, 128 * kt:128 * (kt + 1)],
                    rhs=q8[jt][off:off + 64, qs:qs + cn],
                    start=True, stop=True,
                    tile_position=(off, 0))

        def av(po, pt, hh, kt, groups):
            last = (kt == NT - 1)
            for (g, pc, s) in groups:
                nc.tensor.matmul(
                    po[:, 65 * g:65 * g + 65],
                    lhsT=pt[:, pc:pc + 128],
                    rhs=va[s][kt][:, 65 * hh:65 * hh + 65],
                    start=False, stop=last, skip_group_check=True)

        def normalize(po, hh, t0, n):
            po3 = po[:, 0:65 * n].rearrange("p (g e) -> p g e", g=n)
            rz = work.tile([128, NQT], f32, tag="rz")
            rz3 = rz[:, 0:n].rearrange("p (g e) -> p g e", e=1)
            nc.vector.reciprocal_approx_fast(rz3[:, :, :], po3[:, :, 64:65])
            nc.vector.tensor_tensor(
                o3[:, t0:t0 + n, 64 * hh:64 * hh + 64], po3[:, :, 0:D],
                rz3[:, :, :].to_broadcast([128, n, D]), ALU.mult)

        # ---- emission -----------------------------------------------------
        # prologue: everything needed before stream step (h0, kt0),
        # with the last V units deferred into the first head's stream.
        for jt in range(CT):
            proj_q8(jt)
        for s in "abc":
            for jt in range(CT):
                for part in range(2):
                    proj_k8(s, jt, part)
        LEAD = 4
        for nt in range(NT):
            proj_v("a", nt, True)
        for nt in range(LEAD):
            proj_v("b", nt, True)
            proj_v("c", nt, True)
        bg = [("v", s, nt) for nt in range(LEAD, NT) for s in "bc"]
        bgi = 0

        ot_big = qot.tile([128, CT, L], bf16, tag="qotT", name="otbig", bufs=1)

        def transpose_tile(t):
            eng = nc.sync
            eng.dma_start_transpose(
                ot_big[:, :, 128 * t:128 * (t + 1)], o3[:, t, :])

        def proj_out(ctj, lcs):
            py = psS.tile([128, L], f32, tag="psS")
            for f in range(CT):
                nc.tensor.matmul(
                    py[:, 0:512],
                    lhsT=wp_sb[f][:, 128 * ctj:128 * (ctj + 1)],
                    rhs=ot_big[:, f, lcs:lcs + 512],
                    start=(f == 0), stop=(f == CT - 1),
                    skip_group_check=True)
            ys = work.tile([128, 512], f32, tag="ys")
            nc.vector.tensor_scalar(
                ys[:, 0:512], py[:, 0:512], bias_sb[:, ctj:ctj + 1], None, ALU.add)
            nc.sync.dma_start(
                outT[128 * ctj:128 * (ctj + 1), lcs:lcs + 512], ys[:, 0:512])

        for hh in range(H):
            po1 = psO.tile([128, 65 * 7], f32, tag="po1")
            po2 = psO.tile([128, 65 * 5], f32, tag="po2")
            nc.vector.memset(po1[:], 0.0)
            nc.vector.memset(po2[:], 0.0)
            for kt in range(NT):
                ps = psS.tile([128, L], f32, tag="psS")
                scores(ps, hh, kt)
                pt = pts.tile([128, L], bf16, tag="pt")
                nc.scalar.activation(pt[:, :], ps[:, 0:L], AF.Exp, scale=EXPSCALE)
                av(po1, pt, hh, kt, AV_G1)
                av(po2, pt, hh, kt, AV_G2)
                if bgi < len(bg):
                    _, s_, nt_ = bg[bgi]
                    proj_v(s_, nt_, False)
                    bgi += 1
                    if bgi < len(bg):
                        _, s_, nt_ = bg[bgi]
                        proj_v(s_, nt_, False)
                        bgi += 1
            normalize(po1, hh, 0, 7)
            normalize(po2, hh, 7, 5)

        # all transposes first: out-DMAs share the sync DGE queue, and a
        # waiting out-DMA would block later transposes queued behind it
        for t in range(NQT):
            transpose_tile(t)
        for g in range(3):
            for ctj in range(CT):
                proj_out(ctj, 512 * g)

    nc.compile()
    _BUILD_CACHE["nc"] = nc
    return nc


def _assign(q_lengths):
    """Slot assignment: per core (a_batch, a_len, [(b,s,l), (b,s,l)])."""
    q_lengths = [int(v) for v in q_lengths]
    a_len = [min(v, ALEN) for v in q_lengths]
    chunks = []
    for b in range(B):
        s = a_len[b]
        while s < q_lengths[b]:
            take = min(CHUNK, q_lengths[b] - s)
            chunks.append((b, s, take))
            s += take
    assert len(chunks) <= 2 * NCORES, f"too ragged: {len(chunks)} chunks"
    chunks += [(0, 0, 0)] * (2 * NCORES - len(chunks))
    return a_len, [(chunks[2 * c], chunks[2 * c + 1]) for c in range(NCORES)]


def kernel(x, q, Wq, Wkv, Wproj, bproj, q_lengths, max_q_len):
    import ml_dtypes
    from concourse.bass_utils import run_bass_kernel_spmd

    bf16 = ml_dtypes.bfloat16
    x = np.asarray(x, np.float32)
    q = np.asarray(q, np.float32)
    Wq = np.asarray(Wq, np.float32)
    Wkv = np.asarray(Wkv, np.float32)
    Wproj = np.asarray(Wproj, np.float32)
    bproj = np.asarray(bproj, np.float32)
    q_lengths = np.asarray(q_lengths, np.int64)
    assert x.shape[0] == NCORES == B

    nc = _build()

    offs = np.concatenate([[0], np.cumsum(q_lengths)])
    wqT = np.ascontiguousarray(Wq.T * W8).astype(bf16)
    wkT = np.ascontiguousarray(Wkv[:C].T * W8).astype(bf16)
    wvT = np.ascontiguousarray(Wkv[C:].T).astype(bf16)
    wpT = np.ascontiguousarray(Wproj.T).astype(bf16)
    biasP = np.ascontiguousarray(bproj.reshape(CT, 128).T).astype(np.float32)

    a_len, core_chunks = _assign(q_lengths)
    xTs = [np.ascontiguousarray(x[b].T).astype(bf16) for b in range(B)]

    in_maps = []
    for c in range(NCORES):
        (b1, s1, l1), (b2, s2, l2) = core_chunks[c]
        qTp = np.zeros((C, L), bf16)
        qTp[:, :a_len[c]] = q[offs[c]:offs[c] + a_len[c]].T.astype(bf16)
        if l1:
            qTp[:, ALEN:ALEN + l1] = q[offs[b1] + s1:offs[b1] + s1 + l1].T.astype(bf16)
        if l2:
            qTp[:, ALEN + CHUNK:ALEN + CHUNK + l2] = \
                q[offs[b2] + s2:offs[b2] + s2 + l2].T.astype(bf16)
        in_maps.append({
            "xTa": xTs[c], "xTb": xTs[b1], "xTc": xTs[b2],
            "qT": qTp,
            "wqT": wqT, "wkT": wkT, "wvT": wvT, "wpT": wpT,
            "biasP": biasP,
        })

    trace = os.environ.get("KERNEL_TRACE", "") == "1"
    if trace:
        try:
            import sys
            import types
            import antenv
            if "antenv.axon_hooks" not in sys.modules:
                from trn_agent_boot.trn_boot import _ntff_profile_via_ctypes
                hook = _ntff_profile_via_ctypes("/opt/axon/libaxon_pjrt.so")
                mod = types.ModuleType("antenv.axon_hooks")
                mod.get_axon_ntff_profile_hook = lambda: hook
                sys.modules["antenv.axon_hooks"] = mod
                antenv.axon_hooks = mod
        except Exception as e:
            print(f"ntff hook setup failed: {e}")
            trace = False
    res = run_bass_kernel_spmd(nc, in_maps, core_ids=list(range(NCORES)),
                               trace=trace)
    if trace and res.exec_time_ns is not None:
        print(f"HW exec time: {res.exec_time_ns} ns")
        if res.instructions_and_trace:
            print(f"trace: {res.instructions_and_trace[1]}")

    out = np.empty((int(offs[-1]), C), np.float32)
    for c in range(NCORES):
        yT = res.results[c]["out"]
        out[offs[c]:offs[c] + a_len[c]] = yT[:, :a_len[c]].T
        (b1, s1, l1), (b2, s2, l2) = core_chunks[c]
        if l1:
            out[offs[b1] + s1:offs[b1] + s1 + l1] = yT[:, ALEN:ALEN + l1].T
        if l2:
            out[offs[b2] + s2:offs[b2] + s2 + l2] = \
                yT[:, ALEN + CHUNK:ALEN + CHUNK + l2].T
    return out


# revision 24
# speedup vs baseline: 1.2031x; 1.2031x over previous
"""Trainium2 Bass kernel: varlen batched cross-attention (sparse_attention).

Math (per reference):
  qh = q @ Wq.T           [Tq, H, D]
  k,v = split(x @ Wkv.T)  [B, N, H, D]
  per batch b: queries of segment b attend over batch b's N keys
  out = softmax(qh k^T / sqrt(D)) v  -> [Tq, C] @ Wproj.T + bproj

Design:
  Load-balanced 3-slot sharding: every core processes 1536 query slots
  (12 tiles of 128): slot A = first 1024 queries of its home batch,
  slots B, C = 256-query chunks of longer batches' remainders.  Every
  core computes K,V for its 3 slot batches locally; zero collectives;
  one uniform NEFF; host scatters inputs / gathers outputs.

  Scores run on the PE in plain fp8e4 (64-dim contraction, heads packed
  two-per-128-partition tile, tile_position row offset 0/64).  Wq/Wk
  are host-scaled by 8 for fp8 range (exp scale folds 1/64 back); K^T
  and qh^T are cast f32->fp8 from the projection PSUM on the DVE.
  (fp8 DoubleRow was tried and measured ~2x SLOWER per column than
  plain fp8/bf16 on real hardware, despite the cost model's 0.5
  cycles/row - do not bring it back without re-measuring.)

  exp() on ACT is the stream bottleneck (~1.53us per [128,1536] chunk,
  128 chunks): single-pass over (head, keytile), PSUM chunk ring
  2x[128,1536] (6 banks) + two po accumulators (2 banks) = 8 banks.
  Score matmuls must not cross 512-col PSUM bank boundaries.

  AV uses the P-stationary form: po[qtile, 65] += P~[kt, qtile]^T @
  V_aug[kt]; the V ones-column yields Z for free; 16-keytile PSUM
  accumulation uses memset + start=False multi-group banks.  Normalize
  = DVE recip + broadcast-mult into O [q, feat] bf16; O -> O^T via
  DMA-engine transpose (sync queue ONLY - issuing dma_start_transpose
  from the ACT queue races and corrupts results); y^T = Wp^T O^T +
  bias in 512-col chunks, DMA out [C, 1536] f32 per core.

  Prologue (~110us, ACT idle): all K projections + q8 + V leads; the
  remaining V_b/V_c units drain one-per-window into the first stream
  windows.  Moving MORE projection work into stream windows (v6
  experiment) serialized the stream through the PSUM ring and was
  ~140us slower - the ring FIFO couples background tiles to stream
  chunks, so background work per window must stay under the ~500ns
  ACT slack.
"""

import os
import numpy as np

B, NKEY, C, H, D = 8, 2048, 512, 8, 64
NCORES = 8
CT = C // 128           # 4 c-tiles
NT = NKEY // 128        # 16 key tiles
L = 1536                # uniform padded queries per core
NQT = L // 128          # 12 query tiles
ALEN = 1024             # slot A capacity (tiles 0-7)
CHUNK = 256             # slot B/C capacity (2 tiles each)
SCALE = float(D) ** -0.5
W8 = 8.0                # host pre-scale on Wq/Wk for fp8 range
EXPSCALE = SCALE / (W8 * W8)
P1W = 896               # pass1 width: tiles 0-6
P2W = 640               # pass2 width: tile 7 (128) + B (256) + C (256)

_BUILD_CACHE = {}


def _build():
    if "nc" in _BUILD_CACHE:
        return _BUILD_CACHE["nc"]
    from contextlib import ExitStack
    import concourse.bass as bass
    import concourse.tile as tile
    import concourse.mybir as mybir
    from concourse import bacc

    f32 = mybir.dt.float32
    bf16 = mybir.dt.bfloat16
    fp8 = mybir.dt.float8e4
    AF = mybir.ActivationFunctionType
    ALU = mybir.AluOpType
    DR = mybir.MatmulPerfMode.DoubleRow

    nc = bacc.Bacc("TRN2", target_bir_lowering=False, debug=False)
    xT = {s: nc.declare_dram_parameter(f"xT{s}", [C, NKEY], bf16, isOutput=False)
          for s in "abc"}
    qT = nc.declare_dram_parameter("qT", [C, L], bf16, isOutput=False)
    wqT = nc.declare_dram_parameter("wqT", [C, C], bf16, isOutput=False)   # permuted, x8
    wkT = nc.declare_dram_parameter("wkT", [C, C], bf16, isOutput=False)   # permuted, x8
    wvT = nc.declare_dram_parameter("wvT", [C, C], bf16, isOutput=False)
    wpT = nc.declare_dram_parameter("wpT", [C, C], bf16, isOutput=False)
    biasP = nc.declare_dram_parameter("biasP", [128, CT], f32, isOutput=False)
    outT = nc.declare_dram_parameter("out", [C, L], f32, isOutput=True)

    with ExitStack() as ctx:
        tc = ctx.enter_context(tile.TileContext(nc))
        pers = ctx.enter_context(tc.tile_pool(name="pers", bufs=1))
        psS = ctx.enter_context(tc.tile_pool(name="psS", bufs=2, space="PSUM"))
        psO = ctx.enter_context(tc.tile_pool(name="psO", bufs=1, space="PSUM"))
        pts = ctx.enter_context(tc.tile_pool(name="pts", bufs=3))
        work = ctx.enter_context(tc.tile_pool(name="work", bufs=2))
        qot = ctx.enter_context(tc.tile_pool(name="qot", bufs=4))

        # ---- persistent inputs -------------------------------------------
        wq_sb = [pers.tile([128, C], bf16, tag=f"wq{i}", name=f"wq{i}") for i in range(CT)]
        wk_sb = [pers.tile([128, C], bf16, tag=f"wk{i}", name=f"wk{i}") for i in range(CT)]
        wv_sb = [pers.tile([128, C], bf16, tag=f"wv{i}", name=f"wv{i}") for i in range(CT)]
        wp_sb = [pers.tile([128, C], bf16, tag=f"wp{i}", name=f"wp{i}") for i in range(CT)]
        xs_sb = {s: [pers.tile([128, NKEY], bf16, tag=f"x{s}{i}", name=f"x{s}{i}")
                     for i in range(CT)] for s in "abc"}
        # qt tiles share a rotating pool with OT (disjoint lifetimes)
        qt_sb = [qot.tile([128, L], bf16, tag="qot", name=f"qt{i}") for i in range(CT)]
        bias_sb = pers.tile([128, CT], f32, tag="bias")

        for i in range(CT):
            sl = slice(128 * i, 128 * (i + 1))
            nc.sync.dma_start(wq_sb[i][:], wqT[sl, :])
            nc.sync.dma_start(qt_sb[i][:], qT[sl, :])
            nc.sync.dma_start(wk_sb[i][:], wkT[sl, :])
            nc.sync.dma_start(xs_sb["a"][i][:], xT["a"][sl, :])
            nc.sync.dma_start(wv_sb[i][:], wvT[sl, :])
        for i in range(CT):
            sl = slice(128 * i, 128 * (i + 1))
            nc.sync.dma_start(xs_sb["b"][i][:], xT["b"][sl, :])
            nc.sync.dma_start(xs_sb["c"][i][:], xT["c"][sl, :])
            nc.sync.dma_start(wp_sb[i][:], wpT[sl, :])
        nc.sync.dma_start(bias_sb[:], biasP[:])

        # fp8 operands, natural layout: tile jt = heads (2jt, 2jt+1) x d(64)
        q8 = [pers.tile([128, L], fp8, tag=f"q8{t}", name=f"q8{t}") for t in range(CT)]
        k8 = {s: [pers.tile([128, NKEY], fp8, tag=f"k8{s}{t}", name=f"k8{s}{t}")
                  for t in range(CT)] for s in "abc"}
        va = {s: [pers.tile([128, H * (D + 1)], bf16, tag=f"va{s}{n}", name=f"va{s}{n}")
                  for n in range(NT)] for s in "abc"}

        # ---- projections (psum staged in the shared psS ring) -----------
        def proj_q8(jt):
            ps = psS.tile([128, L], f32, tag="psS")
            for lcs in range(0, L, 512):
                for ct in range(CT):
                    nc.tensor.matmul(
                        ps[:, lcs:lcs + 512],
                        lhsT=wq_sb[ct][:, 128 * jt:128 * (jt + 1)],
                        rhs=qt_sb[ct][:, lcs:lcs + 512],
                        start=(ct == 0), stop=(ct == CT - 1),
                        skip_group_check=True)
            nc.vector.tensor_copy(q8[jt][:, 0:L], ps[:, 0:L])

        def proj_k8(s, jt, part):
            # part 0: key cols 0:1536 (3 groups); part 1: cols 1536:2048
            if part == 0:
                ps = psS.tile([128, L], f32, tag="psS")
                for g in range(3):
                    for ct in range(CT):
                        nc.tensor.matmul(
                            ps[:, 512 * g:512 * (g + 1)],
                            lhsT=wk_sb[ct][:, 128 * jt:128 * (jt + 1)],
                            rhs=xs_sb[s][ct][:, 512 * g:512 * (g + 1)],
                            start=(ct == 0), stop=(ct == CT - 1),
                            skip_group_check=True)
                nc.vector.tensor_copy(k8[s][jt][:, 0:1536], ps[:, 0:1536])
            else:
                ps = psS.tile([128, L], f32, tag="psS")
                for ct in range(CT):
                    nc.tensor.matmul(
                        ps[:, 0:512],
                        lhsT=wk_sb[ct][:, 128 * jt:128 * (jt + 1)],
                        rhs=xs_sb[s][ct][:, 1536:2048],
                        start=(ct == 0), stop=(ct == CT - 1),
                        skip_group_check=True)
                nc.vector.tensor_copy(k8[s][jt][:, 1536:2048], ps[:, 0:512])

        def proj_v(s, nt, on_act):
            ps = psS.tile([128, L], f32, tag="psS")
            for ct in range(CT):
                nc.tensor.matmul(
                    ps[:, 0:512],
                    lhsT=xs_sb[s][ct][:, 128 * nt:128 * (nt + 1)],
                    rhs=wv_sb[ct][:, 0:C],
                    start=(ct == 0), stop=(ct == CT - 1),
                    skip_group_check=True)
            va3 = va[s][nt][:].rearrange("p (h e) -> p h e", h=H)
            ps3 = ps[:, 0:512].rearrange("p (h d) -> p h d", h=H)
            if on_act:
                nc.scalar.copy(va3[:, :, 0:D], ps3[:, :, :])
            else:
                nc.vector.tensor_copy(va3[:, :, 0:D], ps3[:, :, :])
            nc.vector.memset(va3[:, :, D:D + 1], 1.0)

        # ---- attention ----------------------------------------------------
        o_big = pers.tile([128, NQT, C], bf16, tag="obig")
        o3 = o_big[:]

        # single-pass chunk layout (none may cross a 512-col psum bank):
        SC_PARTS = [(0, 512, "a", 0), (512, 512, "a", 512),
                    (1024, 256, "b", 1024), (1280, 256, "c", 1280)]
        AV_G1 = [(g, 128 * g, "a") for g in range(7)]               # tiles 0-6
        AV_G2 = [(0, 896, "a"), (1, 1024, "b"), (2, 1152, "b"),
                 (3, 1280, "c"), (4, 1408, "c")]                    # tiles 7-11

        def scores(ps, hh, kt):
            jt, off = hh // 2, 64 * (hh % 2)
            for (cs, cn, s, qs) in SC_PARTS:
                nc.tensor.matmul(
                    ps[:, cs:cs + cn],
                    lhsT=k8[s][jt][off:off + 64, 128 * kt:128 * (kt + 1)],
                    rhs=q8[jt][off:off + 64, qs:qs + cn],
                    start=True, stop=True,
                    tile_position=(off, 0))

        def av(po, pt, hh, kt, groups):
            last = (kt == NT - 1)
            for (g, pc, s) in groups:
                nc.tensor.matmul(
                    po[:, 65 * g:65 * g + 65],
                    lhsT=pt[:, pc:pc + 128],
                    rhs=va[s][kt][:, 65 * hh:65 * hh + 65],
                    start=False, stop=last, skip_group_check=True)

        def normalize(po, hh, t0, n):
            po3 = po[:, 0:65 * n].rearrange("p (g e) -> p g e", g=n)
            rz = work.tile([128, NQT], f32, tag="rz")
            rz3 = rz[:, 0:n].rearrange("p (g e) -> p g e", e=1)
            nc.vector.reciprocal_approx_fast(rz3[:, :, :], po3[:, :, 64:65])
            nc.vector.tensor_tensor(
                o3[:, t0:t0 + n, 64 * hh:64 * hh + 64], po3[:, :, 0:D],
                rz3[:, :, :].to_broadcast([128, n, D]), ALU.mult)

        # ---- emission -----------------------------------------------------
        # prologue: everything needed before stream step (h0, kt0),
        # with the last V units deferred into the first head's stream.
        for jt in range(CT):
            proj_q8(jt)
        for s in "abc":
            for jt in range(CT):
                for part in range(2):
                    proj_k8(s, jt, part)
        LEAD = 4
        for nt in range(NT):
            proj_v("a", nt, True)
        for nt in range(LEAD):
            proj_v("b", nt, True)
            proj_v("c", nt, True)
        bg = [("v", s, nt) for nt in range(LEAD, NT) for s in "bc"]
        bgi = 0

        ot_big = qot.tile([128, CT, L], bf16, tag="qotT", name="otbig", bufs=1)

        def transpose_tile(t):
            eng = nc.sync
            eng.dma_start_transpose(
                ot_big[:, :, 128 * t:128 * (t + 1)], o3[:, t, :])

        def proj_out(ctj, lcs):
            py = psS.tile([128, L], f32, tag="psS")
            for f in range(CT):
                nc.tensor.matmul(
                    py[:, 0:512],
                    lhsT=wp_sb[f][:, 128 * ctj:128 * (ctj + 1)],
                    rhs=ot_big[:, f, lcs:lcs + 512],
                    start=(f == 0), stop=(f == CT - 1),
                    skip_group_check=True)
            ys = work.tile([128, 512], f32, tag="ys")
            nc.vector.tensor_scalar(
                ys[:, 0:512], py[:, 0:512], bias_sb[:, ctj:ctj + 1], None, ALU.add)
            nc.sync.dma_start(
                outT[128 * ctj:128 * (ctj + 1), lcs:lcs + 512], ys[:, 0:512])

        for hh in range(H):
            po1 = psO.tile([128, 65 * 7], f32, tag="po1")
            po2 = psO.tile([128, 65 * 5], f32, tag="po2")
            nc.vector.memset(po1[:], 0.0)
            nc.vector.memset(po2[:], 0.0)
            for kt in range(NT):
                ps = psS.tile([128, L], f32, tag="psS")
                scores(ps, hh, kt)
                pt = pts.tile([128, L], bf16, tag="pt")
                nc.scalar.activation(pt[:, :], ps[:, 0:L], AF.Exp, scale=EXPSCALE)
                av(po1, pt, hh, kt, AV_G1)
                av(po2, pt, hh, kt, AV_G2)
                if bgi < len(bg):
                    _, s_, nt_ = bg[bgi]
                    proj_v(s_, nt_, False)
                    bgi += 1
                    if bgi < len(bg):
                        _, s_, nt_ = bg[bgi]
                        proj_v(s_, nt_, False)
                        bgi += 1
            normalize(po1, hh, 0, 7)
            normalize(po2, hh, 7, 5)

        # all transposes first: out-DMAs share the sync DGE queue, and a
        # waiting out-DMA would block later transposes queued behind it
        for t in range(NQT):
            transpose_tile(t)
        for g in range(3):
            for ctj in range(CT):
                proj_out(ctj, 512 * g)

    nc.compile()
    _BUILD_CACHE["nc"] = nc
    return nc


def _assign(q_lengths):
    """Slot assignment: per core (a_batch, a_len, [(b,s,l), (b,s,l)])."""
    q_lengths = [int(v) for v in q_lengths]
    a_len = [min(v, ALEN) for v in q_lengths]
    chunks = []
    for b in range(B):
        s = a_len[b]
        while s < q_lengths[b]:
            take = min(CHUNK, q_lengths[b] - s)
            chunks.append((b, s, take))
            s += take
    assert len(chunks) <= 2 * NCORES, f"too ragged: {len(chunks)} chunks"
    chunks += [(0, 0, 0)] * (2 * NCORES - len(chunks))
    return a_len, [(chunks[2 * c], chunks[2 * c + 1]) for c in range(NCORES)]


def kernel(x, q, Wq, Wkv, Wproj, bproj, q_lengths, max_q_len):
    import ml_dtypes
    from concourse.bass_utils import run_bass_kernel_spmd

    bf16 = ml_dtypes.bfloat16
    x = np.asarray(x, np.float32)
    q = np.asarray(q, np.float32)
    Wq = np.asarray(Wq, np.float32)
    Wkv = np.asarray(Wkv, np.float32)
    Wproj = np.asarray(Wproj, np.float32)
    bproj = np.asarray(bproj, np.float32)
    q_lengths = np.asarray(q_lengths, np.int64)
    assert x.shape[0] == NCORES == B

    nc = _build()

    offs = np.concatenate([[0], np.cumsum(q_lengths)])
    wqT = np.ascontiguousarray(Wq.T * W8).astype(bf16)
    wkT = np.ascontiguousarray(Wkv[:C].T * W8).astype(bf16)
    wvT = np.ascontiguousarray(Wkv[C:].T).astype(bf16)
    wpT = np.ascontiguousarray(Wproj.T).astype(bf16)
    biasP = np.ascontiguousarray(bproj.reshape(CT, 128).T).astype(np.float32)

    a_len, core_chunks = _assign(q_lengths)
    xTs = [np.ascontiguousarray(x[b].T).astype(bf16) for b in range(B)]

    in_maps = []
    for c in range(NCORES):
        (b1, s1, l1), (b2, s2, l2) = core_chunks[c]
        qTp = np.zeros((C, L), bf16)
        qTp[:, :a_len[c]] = q[offs[c]:offs[c] + a_len[c]].T.astype(bf16)
        if l1:
            qTp[:, ALEN:ALEN + l1] = q[offs[b1] + s1:offs[b1] + s1 + l1].T.astype(bf16)
        if l2:
            qTp[:, ALEN + CHUNK:ALEN + CHUNK + l2] = \
                q[offs[b2] + s2:offs[b2] + s2 + l2].T.astype(bf16)
        in_maps.append({
            "xTa": xTs[c], "xTb": xTs[b1], "xTc": xTs[b2],
            "qT": qTp,
            "wqT": wqT, "wkT": wkT, "wvT": wvT, "wpT": wpT,
            "biasP": biasP,
        })

    trace = os.environ.get("KERNEL_TRACE", "") == "1"
    if trace:
        try:
            import sys
            import types
            import antenv
            if "antenv.axon_hooks" not in sys.modules:
                from trn_agent_boot.trn_boot import _ntff_profile_via_ctypes
                hook = _ntff_profile_via_ctypes("/opt/axon/libaxon_pjrt.so")
                mod = types.ModuleType("antenv.axon_hooks")
                mod.get_axon_ntff_profile_hook = lambda: hook
                sys.modules["antenv.axon_hooks"] = mod
                antenv.axon_hooks = mod
        except Exception as e:
            print(f"ntff hook setup failed: {e}")
            trace = False
    res = run_bass_kernel_spmd(nc, in_maps, core_ids=list(range(NCORES)),
                               trace=trace)
    if trace and res.exec_time_ns is not None:
        print(f"HW exec time: {res.exec_time_ns} ns")
        if res.instructions_and_trace:
            print(f"trace: {res.instructions_and_trace[1]}")

    out = np.empty((int(offs[-1]), C), np.float32)
    for c in range(NCORES):
        yT = res.results[c]["out"]
        out[offs[c]:offs[c] + a_len[c]] = yT[:, :a_len[c]].T
        (b1, s1, l1), (b2, s2, l2) = core_chunks[c]
        if l1:
            out[offs[b1] + s1:offs[b1] + s1 + l1] = yT[:, ALEN:ALEN + l1].T
        if l2:
            out[offs[b2] + s2:offs[b2] + s2 + l2] = \
                yT[:, ALEN + CHUNK:ALEN + CHUNK + l2].T
    return out
